# revision 11
# baseline (speedup 1.0000x reference)
"""GAT-style attention score kernel for 8 TRN2 NeuronCores.

Computes out[i,j] = LeakyReLU(Wh[i]@a1 + Wh[j]@a2, slope=0.2) for
N=8192, D=64 -> [8192, 8192] f32 output (256MB).

Sharding: output rows across 8 cores ([1024, 8192] slab each). Each core
gets the full transposed Wh (replicated) + its row slice, f16 for the
tiny matmuls (error ~1e-3 abs, output is f32 throughout after PSUM).

Per-core pipeline (memory-bound: the 32MB output write is the wall):
  PE:     s2 broadcast tile s2b[p,f] = s2[f] via matmuls with an
          a2-replicated stationary [64,128]; s1 via whTr-stationary
          matmuls -> [128,8]. Warmup dummy matmuls ramp the PE clock.
  Scalar: copy PSUM->SBUF (s2b quarters, s1), then per-quarter
          x = Identity(s2b + s1[k]) (bias add) into ping-pong x bufs.
  Vector: out = max(0.2*x, x) (exact LeakyReLU; HW Lrelu slope is
          hardwired to 0.01 so the activation path is unusable).
  Sync:   output DMA stream - tile 0 in quarters to start the wall
          early, then full 4MB tiles at ~fabric rate.
"""

import numpy as np
import concourse.bass as bass
import concourse.mybir as mybir
from concourse.bass_utils import run_bass_kernel_spmd

N = 8192          # nodes
D = 64            # feature dim
M = 8             # cores
ROWS = N // M     # 1024 output rows per core
NT = ROWS // 128  # 8 row tiles of 128 partitions
FCH = 512         # matmul moving-dim chunk
QW = 2048         # quarter width
NG = NT * 4       # 32 quarter groups
NEG_SLOPE = 0.2
N_WARM = 10       # dummy matmuls to ramp PE pstate

_cache = {}


def _build():
    nc = bass.Bass()
    f32 = mybir.dt.float32
    f16 = mybir.dt.float16

    whT_ext = nc.declare_dram_parameter("whT", [D, N], f16, isOutput=False)
    whTr_ext = nc.declare_dram_parameter("whTr", [D, ROWS], f16, isOutput=False)
    a1_ext = nc.declare_dram_parameter("a1", [D, 1], f16, isOutput=False)
    a2r_ext = nc.declare_dram_parameter("a2r", [D, 128], f16, isOutput=False)
    out_ext = nc.declare_dram_parameter("out", [ROWS, N], f32, isOutput=True)

    from contextlib import ExitStack
    with ExitStack() as ctx:
        sb_whT = ctx.enter_context(nc.sbuf_tensor("sb_whT", [D, N], f16))
        sb_whTr = ctx.enter_context(nc.sbuf_tensor("sb_whTr", [D, ROWS], f16))
        sb_a1 = ctx.enter_context(nc.sbuf_tensor("sb_a1", [D, 1], f16))
        sb_a2r = ctx.enter_context(nc.sbuf_tensor("sb_a2r", [D, 128], f16))
        sb_s1 = ctx.enter_context(nc.sbuf_tensor("sb_s1", [128, NT], f32))
        sb_s2b = ctx.enter_context(nc.sbuf_tensor("sb_s2b", [128, N], f32))
        sb_xq0 = ctx.enter_context(nc.sbuf_tensor("sb_xq0", [128, QW], f32))
        sb_xq1 = ctx.enter_context(nc.sbuf_tensor("sb_xq1", [128, QW], f32))
        sb_o0 = ctx.enter_context(nc.sbuf_tensor("sb_o0", [128, N], f32))
        sb_o1 = ctx.enter_context(nc.sbuf_tensor("sb_o1", [128, N], f32))
        sb_junk = ctx.enter_context(nc.sbuf_tensor("sb_junk", [128, 1], f32))
        ps_a = ctx.enter_context(nc.psum_tensor("ps_a", [128, QW], f32))
        ps_b = ctx.enter_context(nc.psum_tensor("ps_b", [128, QW], f32))
        din = ctx.enter_context(nc.semaphore("din"))
        dwh = ctx.enter_context(nc.semaphore("dwh"))
        mm = ctx.enter_context(nc.semaphore("mm"))
        scp = ctx.enter_context(nc.semaphore("scp"))
        cq = ctx.enter_context(nc.semaphore("cq"))
        xs = ctx.enter_context(nc.semaphore("xs"))
        sst = ctx.enter_context(nc.semaphore("sst"))
        dout = ctx.enter_context(nc.semaphore("dout"))
        block = ctx.enter_context(nc.Block())
        sb_xq = [sb_xq0, sb_xq1]
        sb_o = [sb_o0, sb_o1]
        ps = [ps_a, ps_b]

        @block.sync
        def _(sync):
            sync.dma_start(sb_whT[:, 0:QW], whT_ext[:, 0:QW]).then_inc(dwh, 16)
            sync.dma_start(sb_a2r[:, :], a2r_ext[:, :]).then_inc(din, 16)
            sync.dma_start(sb_a1[:, :], a1_ext[:, :]).then_inc(din, 16)
            sync.dma_start(sb_whTr[:, :], whTr_ext[:, :]).then_inc(din, 16)
            for c in range(1, 4):
                sync.dma_start(
                    sb_whT[:, c * QW:(c + 1) * QW], whT_ext[:, c * QW:(c + 1) * QW]
                ).then_inc(dwh, 16)
            # tile 0 in quarters to start the output wall early
            for q in range(4):
                sync.wait_ge(sst, q + 1)
                sync.dma_start(
                    out_ext[0:128, q * QW:(q + 1) * QW],
                    sb_o0[:, q * QW:(q + 1) * QW],
                ).then_inc(dout, 16)
            for k in range(1, NT):
                sync.wait_ge(sst, 4 * k + 4)
                sync.dma_start(
                    out_ext[k * 128:(k + 1) * 128, :], sb_o[k % 2][:, :]
                ).then_inc(dout, 16)

        @block.tensor
        def _(tensor):
            # ramp the PE clock on garbage data while inputs stream in
            for w in range(N_WARM):
                tensor.matmul(
                    ps_b[:, (w % 4) * FCH:(w % 4 + 1) * FCH],
                    sb_whTr[:, 0:128],
                    sb_whT[:, 0:FCH],
                )
            # s2b quarter 0: mm 1-4
            tensor.wait_ge(din, 16)
            tensor.wait_ge(dwh, 16)
            for j in range(4):
                tensor.matmul(
                    ps_a[:, j * FCH:(j + 1) * FCH],
                    sb_a2r[:, :],
                    sb_whT[:, j * FCH:(j + 1) * FCH],
                ).then_inc(mm)
            # s1 into ps_b cols 0..7: mm 5-12
            tensor.wait_ge(din, 48)
            for k in range(NT):
                tensor.matmul(
                    ps_b[:, k:k + 1],
                    sb_whTr[:, k * 128:(k + 1) * 128],
                    sb_a1[:, :],
                ).then_inc(mm)
            # s2b quarters 1-3: mm 13-24 (psum b, a, b)
            for qq in range(1, 4):
                tensor.wait_ge(dwh, 16 * (qq + 1))
                if qq == 1:
                    tensor.wait_ge(scp, 1)      # s1 copied out of ps_b
                else:
                    tensor.wait_ge(cq, qq - 1)  # psum buf drained by quarter copy
                for j in range(4 * qq, 4 * qq + 4):
                    tensor.matmul(
                        ps[qq % 2][:, (j % 4) * FCH:(j % 4 + 1) * FCH],
                        sb_a2r[:, :],
                        sb_whT[:, j * FCH:(j + 1) * FCH],
                    ).then_inc(mm)

        @block.scalar
        def _(scalar):
            # warm the activation table while input DMAs fly
            scalar.activation(
                sb_junk[:, :], sb_junk[:, :],
                mybir.ActivationFunctionType.Identity,
                bias=sb_junk[:, 0:1], scale=1.0,
            )
            scalar.wait_ge(mm, 12)
            scalar.copy(sb_s1[:, :], ps_b[:, 0:NT]).then_inc(scp)
            for qq in range(4):
                scalar.wait_ge(mm, 4 if qq == 0 else 12 + 4 * qq)
                scalar.copy(
                    sb_s2b[:, qq * QW:(qq + 1) * QW], ps[qq % 2][:, :]
                ).then_inc(cq)
            for g in range(NG):
                k, q = g // 4, g % 4
                if k == 0:
                    scalar.wait_ge(cq, q + 1)
                elif k == 1 and q == 0:
                    scalar.wait_ge(cq, 4)
                if g >= 2:
                    scalar.wait_ge(sst, g - 1)  # x buf g%2 consumed
                scalar.activation(
                    sb_xq[g % 2][:, :],
                    sb_s2b[:, q * QW:(q + 1) * QW],
                    mybir.ActivationFunctionType.Identity,
                    bias=sb_s1[:, k:k + 1], scale=1.0,
                ).then_inc(xs)

        @block.vector
        def _(vector):
            for g in range(NG):
                k, q = g // 4, g % 4
                vector.wait_ge(xs, g + 1)
                if q == 0 and k >= 2:
                    vector.wait_ge(dout, 64 if k == 2 else 64 + 16 * (k - 2))
                vector.scalar_tensor_tensor(
                    sb_o[k % 2][:, q * QW:(q + 1) * QW],
                    sb_xq[g % 2][:, :],
                    NEG_SLOPE,
                    sb_xq[g % 2][:, :],
                    mybir.AluOpType.mult,
                    mybir.AluOpType.max,
                ).then_inc(sst)

    return nc


def _run(Wh, a, trace=False, **kw):
    Wh = np.ascontiguousarray(np.asarray(Wh, dtype=np.float32))
    a = np.ascontiguousarray(np.asarray(a, dtype=np.float32))
    assert Wh.shape == (N, D) and a.shape == (2 * D, 1)

    if "nc" not in _cache:
        _cache["nc"] = _build()
    nc = _cache["nc"]

    WhT16 = np.ascontiguousarray(Wh.T.astype(np.float16))        # [64, 8192]
    a1 = np.ascontiguousarray(a[:D, :].astype(np.float16))       # [64, 1]
    a2r = np.ascontiguousarray(np.tile(a[D:, :].astype(np.float16), (1, 128)))
    in_maps = [
        {
            "whT": WhT16,
            "whTr": np.ascontiguousarray(WhT16[:, i * ROWS:(i + 1) * ROWS]),
            "a1": a1,
            "a2r": a2r,
        }
        for i in range(M)
    ]
    res = run_bass_kernel_spmd(nc, in_maps, core_ids=list(range(M)), trace=trace, **kw)
    out = np.concatenate([res.results[i]["out"] for i in range(M)], axis=0)
    return out, res


def kernel(Wh, a):
    return _run(Wh, a)[0]
